# revision 1
# baseline (speedup 1.0000x reference)
"""AGCN (graph conv net block) Trainium2 kernel, 8-core batch-parallel.

Problem: nn_Agcn_87058987090108
  x [32, 32, 512, 64] (B, C, V, L) fp32
  adjacency chain: A_j = softmax(relu(n1_j @ n2_j), axis=1), nodevecs updated
  by tiny 10x10 transforms between hops; 3 graph convs over V with L-rolls;
  concat [x, x1, x2, x3] -> 1x1 conv (128 -> 64 channels) + bias.

Per-core design (4 batches/core, everything else replicated):
  - adjacency chain computed on-chip per core (tiny).
  - conv hop j: psum[w128, (l16, c32)] += A_j[vc,:wc].T @ piece_{j-1}[vc, chunk]
    in float32r (full PE rate, ~1e-4 matmul precision). Rolls = read-offset
    into a zero-padded L prefix of the piece tiles.
  - each conv psum is evicted twice:
      ACT copy  -> pv (f32r, (l,c)-major) chain layout for the next hop
      DVE 32x32 stream-transpose -> Tf (f32) -> GPSIMD cast -> T (bf16)
    giving T[wblk*32+c, wsub*64+l] tiles whose 32-partition slices are pure
    channel ranges.
  - piece 0 (x itself) T-tiles are DMA'd straight from DRAM with bf16 cast.
  - MLP: bf16 matmuls k=32 packed 8-up with tile_position (4 row groups x 2
    col groups, two w-blocks per psum bank), pieces accumulated in psum;
    bias added during eviction; output staged [128=(k2,o), (wsub32, l64)]
    so the final DMA writes contiguous 8KB runs.
"""
import sys
import functools

sys.path.insert(0, '/root/.axon_site/_ro/trn_rl_repo')

import numpy as np

B, C, V, L = 32, 32, 512, 64
E_DIM, KERNEL_SIZE, C_OUT = 10, 3, 64
NCORES = 8
BL = B // NCORES          # batches per core
NVC = V // 128            # 128-row chunks of the node dim
CL = C * L                # 2048
PAD = 64                  # zero prefix (elements) for L-rolls, = 2 l-slots * 32


@functools.lru_cache(maxsize=8)
def _build(variant="full", repeat=1):
    do_conv = variant not in ("nomlp_noconv", "noconv")
    do_mlp = variant not in ("convonly", "convmm", "convpv")
    do_outdma = variant not in ("nooutdma", "convonly", "convmm", "convpv")
    do_tside = variant not in ("notside", "convmm", "convpv")
    do_pv = variant not in ("convmm",)
    do_t0 = do_mlp
    old_hop0 = "oldhop0" in variant
    old_out = "oldout" in variant
    psum8 = "psum8" in variant
    stage6 = "stage6" in variant
    xt = "xt" in variant
    dve_stage = "dvestage" in variant
    cast_eng = "dve" if "dcast" in variant else ("act" if "acast" in variant else "gp")
    t0early = "t0early" in variant
    mixevict = "mixevict" in variant
    t0chip = "t0chip" in variant
    ring2 = "ring2" in variant
    wide = "wide" in variant
    import concourse.bacc as bacc
    import concourse.tile as tile
    from concourse import mybir

    f32 = mybir.dt.float32
    f32r = mybir.dt.float32r
    bf16 = mybir.dt.bfloat16
    AF = mybir.ActivationFunctionType
    AX = mybir.AxisListType
    from concourse.alu_op_type import AluOpType as ALU

    nc = bacc.Bacc("TRN2", target_bir_lowering=False, debug=False)
    x_d = nc.dram_tensor("x", [BL, C, V, L], f32, kind="ExternalInput")
    n1_d = nc.dram_tensor("nodevec1", [V, E_DIM], f32, kind="ExternalInput")
    n2_d = nc.dram_tensor("nodevec2", [E_DIM, V], f32, kind="ExternalInput")
    wt_d = nc.dram_tensor("w_trans", [KERNEL_SIZE - 1, E_DIM, E_DIM], f32, kind="ExternalInput")
    bt_d = nc.dram_tensor("b_trans", [KERNEL_SIZE - 1, E_DIM], f32, kind="ExternalInput")
    mw_d = nc.dram_tensor("mlp_w", [C_OUT, 4 * C], f32, kind="ExternalInput")
    mb_d = nc.dram_tensor("mlp_b", [C_OUT], f32, kind="ExternalInput")
    out_d = nc.dram_tensor("out", [BL, C_OUT, V, L], f32, kind="ExternalOutput")

    xap, outap = x_d.ap(), out_d.ap()

    with tile.TileContext(nc) as tc:
        with (
            tc.tile_pool(name="const", bufs=1) as const_pool,
            tc.tile_pool(name="Amat", bufs=4) as A_pool,
            tc.tile_pool(name="xv", bufs=5 if xt else 4) as xv_pool,
            tc.tile_pool(name="pv", bufs=7) as pv_pool,
            tc.tile_pool(name="Tf", bufs=2) as Tf_pool,
            tc.tile_pool(name="T", bufs=16 if t0early else (13 if (stage6 or xt) else 14)) as T_pool,
            tc.tile_pool(name="stage", bufs=(5 if stage6 else (2 if xt else 2))) as stage_pool,
            tc.tile_pool(name="cps", bufs=(2 if wide else (8 if psum8 else 4)), space="PSUM") as cps_pool,
            tc.tile_pool(name="mps", bufs=(2 if wide else (1 if psum8 else 4)), space="PSUM") as mps_pool,
        ):
            # ---------------- constants / weights ----------------
            # MLP weight stacks per piece j: Wt_j[32k + c, o] = mlp_w[o, 32j + c]
            Wts = []
            for j in range(4):
                Wt = const_pool.tile([128, C_OUT], bf16, name=f"Wt{j}", tag=f"Wt{j}")
                for k in range(4):
                    nc.gpsimd.dma_start(
                        out=Wt[32 * k:32 * k + 32, :],
                        in_=mw_d.ap()[:, 32 * j:32 * j + 32].transpose([1, 0]))
                Wts.append(Wt)
            # bias stacked for the two col-group halves: [128, 1]
            mbt = const_pool.tile([128, 1], f32, name="mbt", tag="mbt")
            for h in range(2):
                nc.gpsimd.dma_start(out=mbt[64 * h:64 * h + 64, :], in_=mb_d.ap().unsqueeze(1))

            # nodevecs as [E, V] f32r; w_trans as [E, E] f32r; b_trans as [E,1] f32
            n1T = const_pool.tile([E_DIM, V], f32r, name="n1T", tag="n1T")
            nc.gpsimd.dma_start(out=n1T[:], in_=n1_d.ap().transpose([1, 0]))
            n2t = const_pool.tile([E_DIM, V], f32r, name="n2t", tag="n2t")
            nc.gpsimd.dma_start(out=n2t[:], in_=n2_d.ap())
            wts_t, bts_t = [], []
            for i in range(KERNEL_SIZE - 1):
                w_t = const_pool.tile([E_DIM, E_DIM], f32r, name=f"wtr{i}", tag=f"wtr{i}")
                nc.gpsimd.dma_start(out=w_t[:], in_=wt_d.ap()[i])
                b_t = const_pool.tile([E_DIM, 1], f32, name=f"btr{i}", tag=f"btr{i}")
                nc.gpsimd.dma_start(out=b_t[:], in_=bt_d.ap()[i].unsqueeze(1))
                wts_t.append(w_t)
                bts_t.append(b_t)

            # ---------------- adjacency chain ----------------
            sm_ctx = tc.tile_pool(name="sm", bufs=1)
            sm_pool = sm_ctx.__enter__()
            A_tiles = []  # A_tiles[j][vc]: [128, V] f32r, rows = v, cols = w
            cur_n1T, cur_n2 = n1T, n2t
            for j in range(KERNEL_SIZE):
                Aj = []
                for vc in range(NVC):
                    zps = (cps_pool if psum8 else mps_pool).tile(
                        [128, V], f32, name=f"zps{j}_{vc}", tag="cps" if psum8 else "mps")
                    nc.tensor.matmul(zps[:], cur_n1T[:, 128 * vc:128 * (vc + 1)],
                                     cur_n2[:], start=True, stop=True)
                    zrelu = sm_pool.tile([128, V], f32, name=f"zrelu{j}_{vc}", tag="zrelu")
                    nc.scalar.activation(zrelu[:], zps[:], AF.Relu)
                    negmax = sm_pool.tile([128, 1], f32, name=f"negmax{j}_{vc}", tag="negmax")
                    nc.vector.reduce_max(negmax[:], zrelu[:], AX.X, negate=True)
                    esum = sm_pool.tile([128, 1], f32, name=f"esum{j}_{vc}", tag="esum")
                    ez = sm_pool.tile([128, V], f32, name=f"ez{j}_{vc}", tag="ez")
                    nc.scalar.activation(ez[:], zrelu[:], AF.Exp, bias=negmax[:],
                                         accum_out=esum[:])
                    rcp = sm_pool.tile([128, 1], f32, name=f"rcp{j}_{vc}", tag="rcp")
                    nc.vector.reciprocal(rcp[:], esum[:])
                    At = A_pool.tile([128, V], f32r, name=f"A{j}_{vc}", tag=f"A{j}")
                    nc.scalar.activation(At[:], ez[:], AF.Identity, scale=rcp[:])
                    Aj.append(At)
                A_tiles.append(Aj)
                if j < KERNEL_SIZE - 1:
                    nn1 = const_pool.tile([E_DIM, V], f32r, name=f"n1T_{j + 1}", tag=f"n1T_{j + 1}")
                    ps = (cps_pool if psum8 else mps_pool).tile(
                        [E_DIM, V], f32, name=f"n1ps{j}", tag="cps" if psum8 else "mps")
                    nc.tensor.matmul(ps[:], wts_t[j][:], cur_n1T[:], start=True, stop=True)
                    nc.scalar.activation(nn1[:], ps[:], AF.Identity, bias=bts_t[j][:])
                    nn2 = const_pool.tile([E_DIM, V], f32r, name=f"n2_{j + 1}", tag=f"n2_{j + 1}")
                    ps2 = (cps_pool if psum8 else mps_pool).tile(
                        [E_DIM, V], f32, name=f"n2ps{j}", tag="cps" if psum8 else "mps")
                    nc.tensor.matmul(ps2[:], wts_t[j][:], cur_n2[:], start=True, stop=True)
                    nc.scalar.activation(nn2[:], ps2[:], AF.Identity, bias=bts_t[j][:])
                    cur_n1T, cur_n2 = nn1, nn2
            sm_ctx.__exit__(None, None, None)

            # ---------------- main loop over local batches ----------------
            def conv_hop(b, j, src_xv, src_pv, shift, store_pv, src_pad=PAD):
                """One graph-conv hop. Returns (pv_tiles or None, T_tiles).

                Hop 0 (src_xv): rhs chunks are contiguous (c8, l64) slices of the
                (c,l)-major x tiles, so psum free = (c8, l64); pv is filled with a
                strided ACT write and T comes from a pv transpose. Hops 1/2: rhs
                chunks are contiguous (l16, c32) slices of (l,c)-major pv tiles;
                T transposes straight off the psum."""
                pvs, Ts = [], []
                for wc in range(NVC):
                    pv = None
                    if (store_pv or src_xv is not None) and do_pv:
                        pv = pv_pool.tile([128, PAD + CL], f32r,
                                          name=f"pv{j}_{b}_{wc}", tag="pv")
                        nc.gpsimd.memset(pv[:, 0:PAD].bitcast(f32), 0.0)
                    T_t = None
                    if do_tside:
                        T_t = T_pool.tile([128, 32 * L], bf16, name=f"T{j}_{b}_{wc}", tag="T")
                    ps_pair = None
                    for q in range(4):
                        if wide:
                            if q % 2 == 0:
                                ps_pair = cps_pool.tile([128, 1024], f32,
                                                        name=f"cps{j}_{b}_{wc}_{q}", tag="cps")
                            ps = ps_pair[:, 512 * (q % 2):512 * (q % 2) + 512]
                        else:
                            ps = cps_pool.tile([128, 512], f32,
                                               name=f"cps{j}_{b}_{wc}_{q}", tag="cps")[:]
                        for vc in range(NVC):
                            if src_xv is not None and old_hop0:
                                rhs = src_xv[vc][:].rearrange("p (c l) -> p l c", c=C)[
                                    :, 16 * q:16 * q + 16, :]
                            elif src_xv is not None:
                                rhs = src_xv[vc][:, 512 * q:512 * (q + 1)]
                            else:
                                off = src_pad + 512 * q - 32 * shift
                                rhs = src_pv[vc][:, off:off + 512]
                            nc.tensor.matmul(ps,
                                             A_tiles[j][vc][:, 128 * wc:128 * (wc + 1)],
                                             rhs, start=(vc == 0), stop=(vc == NVC - 1))
                        if src_xv is not None and not old_hop0:
                            if do_pv:
                                # psum stream (c8, l64) -> pv (l,c)-major strided write
                                dstv = pv[:].rearrange("p (l c) -> p c l", c=C)[
                                    :, 8 * q:8 * q + 8, 2:2 + L]
                                nc.scalar.activation(dstv, ps.rearrange(
                                    "p (c l) -> p c l", c=8), AF.Copy)
                        else:
                            if store_pv and do_pv:
                                if wide:
                                    if q % 2 == 1:
                                        nc.scalar.activation(
                                            pv[:, PAD + 512 * (q - 1):PAD + 512 * (q + 1)],
                                            ps_pair[:], AF.Copy)
                                else:
                                    nc.scalar.activation(
                                        pv[:, PAD + 512 * q:PAD + 512 * (q + 1)],
                                        ps, AF.Copy)
                            if do_tside:
                                Tf = Tf_pool.tile([128, 512], f32,
                                                  name=f"Tf{j}_{b}_{wc}_{q}", tag="Tf")
                                nc.vector.transpose(
                                    Tf[:].rearrange("p (l w) -> p l w", l=16), ps)
                                dst = T_t[:].rearrange("p (w l) -> p l w", w=32)[
                                    :, 16 * q:16 * q + 16, :]
                                if cast_eng == "dve":
                                    nc.vector.tensor_copy(
                                        dst, Tf[:].rearrange("p (l w) -> p l w", l=16))
                                elif cast_eng == "act":
                                    nc.scalar.activation(
                                        dst, Tf[:].rearrange("p (l w) -> p l w", l=16),
                                        AF.Copy)
                                else:
                                    nc.gpsimd.tensor_copy(
                                        dst, Tf[:].rearrange("p (l w) -> p l w", l=16))
                    if src_xv is not None and not old_hop0 and do_tside:
                        # T from pv: natural (l,c) stream, full tile in 4 q-parts
                        for q in range(4):
                            Tf = Tf_pool.tile([128, 512], f32,
                                              name=f"Tf{j}_{b}_{wc}_{q}", tag="Tf")
                            nc.vector.transpose(
                                Tf[:].rearrange("p (l w) -> p l w", l=16),
                                pv[:, PAD + 512 * q:PAD + 512 * (q + 1)].bitcast(f32))
                            dst = T_t[:].rearrange("p (w l) -> p l w", w=32)[
                                :, 16 * q:16 * q + 16, :]
                            nc.gpsimd.tensor_copy(
                                dst, Tf[:].rearrange("p (l w) -> p l w", l=16))
                    pvs.append(pv)
                    Ts.append(T_t)
                return pvs, Ts

            for b4 in range(BL * repeat):
                b = b4 % BL
                # x chunk tiles [v128, (c32, l64)] f32r
                xv = []
                for vc in range(NVC):
                    t = xv_pool.tile([128, CL], f32r, name=f"xv{b4}_{vc}", tag="xv")
                    nc.gpsimd.dma_start(
                        out=t[:].rearrange("p (c l) -> p c l", c=C),
                        in_=xap[b, :, 128 * vc:128 * (vc + 1), :].transpose([1, 0, 2]))
                    xv.append(t)

                T0s = []
                if do_t0 and t0early:
                    for wc in range(NVC):
                        T0 = T_pool.tile([128, 32 * L], bf16, name=f"T0_{b4}_{wc}", tag="T")
                        for wb in range(4):
                            s = xap[b, :, 128 * wc + 32 * wb:128 * wc + 32 * wb + 32, :]
                            nc.gpsimd.dma_start(
                                out=T0[32 * wb:32 * wb + 32, :].rearrange(
                                    "p (ws l) -> p ws l", ws=32),
                                in_=s)
                        T0s.append(T0)

                if do_conv:
                    if xt:
                        xv2 = []
                        for vc in range(NVC):
                            t2 = xv_pool.tile([128, CL], f32r,
                                              name=f"xv2_{b4}_{vc}", tag="xv")
                            nc.vector.tensor_copy(
                                t2[:].rearrange("p (l c) -> p l c", c=C),
                                xv[vc][:].rearrange("p (c l) -> p l c", c=C))
                            xv2.append(t2)
                        p1, T1s = conv_hop(b, 0, None, xv2, 0, True, src_pad=0)
                    else:
                        p1, T1s = conv_hop(b, 0, xv, None, 0, True)
                    if do_pv:
                        p2, T2s = conv_hop(b, 1, None, p1, 1, True)
                        _, T3s = conv_hop(b, 2, None, p2, 2, False)
                    else:  # timing-only: all hops read xv
                        p2, T2s = conv_hop(b, 1, xv, None, 0, True)
                        _, T3s = conv_hop(b, 2, xv, None, 0, False)

                # piece 0 T tiles straight from DRAM (bf16 cast), loaded late to
                # keep the T pool's live set small
                if not do_t0:
                    continue
                if not t0early:
                    for wc in range(NVC):
                        T0 = T_pool.tile([128, 32 * L], bf16,
                                         name=f"T0_{b4}_{wc}", tag="T")
                        if t0chip:
                            # transpose straight off the x tiles: strided (l,c) view
                            for q in range(4):
                                Tf = Tf_pool.tile([128, 512], f32,
                                                  name=f"Tf0_{b4}_{wc}_{q}", tag="Tf")
                                srcv = xv[wc][:].bitcast(f32).rearrange(
                                    "p (c l) -> p l c", c=C)[:, 16 * q:16 * q + 16, :]
                                nc.vector.transpose(
                                    Tf[:].rearrange("p (l w) -> p l w", l=16), srcv)
                                dst = T0[:].rearrange("p (w l) -> p l w", w=32)[
                                    :, 16 * q:16 * q + 16, :]
                                nc.gpsimd.tensor_copy(
                                    dst, Tf[:].rearrange("p (l w) -> p l w", l=16))
                        else:
                            for wb in range(4):
                                s = xap[b, :, 128 * wc + 32 * wb:128 * wc + 32 * wb + 32, :]
                                nc.gpsimd.dma_start(
                                    out=T0[32 * wb:32 * wb + 32, :].rearrange(
                                        "p (ws l) -> p ws l", ws=32),
                                    in_=s)
                        T0s.append(T0)
                pieces = [T0s, T1s, T2s, T3s] if do_conv else [T0s, T0s, T0s, T0s]
                if not do_mlp:
                    continue

                # ---- MLP + output ----
                for wc in range(NVC):
                    for m in range(2):  # pair of w-blocks {2m, 2m+1}
                        if stage6 or xt or t0early:
                            stages = [stage_pool.tile([128, 1024], f32,
                                      name=f"st{b4}_{wc}_{m}_{h}", tag="stage")
                                      for h in range(2)]
                        else:
                            stage = stage_pool.tile([128, 2048], f32,
                                                    name=f"st{b4}_{wc}_{m}", tag="stage")
                        ps3_pair = None
                        for q2 in range(4):  # w_sub chunk [8q2, 8q2+8)
                            if wide:
                                if q2 % 2 == 0:
                                    ps3_pair = mps_pool.tile(
                                        [128, 1024], f32,
                                        name=f"mps{b4}_{wc}_{m}_{q2}", tag="mps")
                                ps3 = ps3_pair[:, 512 * (q2 % 2):512 * (q2 % 2) + 512]
                            else:
                                ps3 = (cps_pool if psum8 else mps_pool).tile(
                                    [128, 512], f32, name=f"mps{b4}_{wc}_{m}_{q2}",
                                    tag="cps" if psum8 else "mps")[:]
                            for k2 in range(2):
                                k = 2 * m + k2
                                for j in range(4):
                                    rhs = pieces[j][wc][32 * k:32 * k + 32,
                                                        512 * q2:512 * (q2 + 1)]
                                    nc.tensor.matmul(
                                        ps3[64 * k2:64 * k2 + 64, :],
                                        Wts[j][32 * k:32 * k + 32, :], rhs,
                                        start=(j == 0), stop=(j == 3),
                                        tile_position=(32 * k, 64 * k2))
                                    # (ps3 is an AP slice; indexing yields sub-AP)
                            if stage6 or xt or t0early:
                                sdst = stages[q2 // 2][:, 512 * (q2 % 2):512 * (q2 % 2 + 1)]
                            else:
                                sdst = stage[:, 512 * q2:512 * (q2 + 1)]
                            use_dve = dve_stage or (mixevict and q2 % 2 == 1)
                            if wide:
                                if q2 % 2 == 1:
                                    wdst = (stage[:, 512 * (q2 - 1):512 * (q2 + 1)]
                                            if not (stage6 or xt or t0early) else None)
                                    if wdst is None:
                                        wdst = stages[q2 // 2][:]
                                    nc.scalar.activation(wdst, ps3_pair[:],
                                                         AF.Identity, bias=mbt[:])
                            elif use_dve:
                                nc.vector.scalar_tensor_tensor(
                                    sdst, ps3[:], 1.0, mbt[:].broadcast_to((128, 512)),
                                    op0=ALU.mult, op1=ALU.add)
                            else:
                                nc.scalar.activation(sdst, ps3[:], AF.Identity,
                                                     bias=mbt[:])
                        # w = 128wc + 64m + 32k2 + ws
                        base = 128 * wc + 64 * m
                        dstf = outap[b, :, base:base + 64, :] \
                            .rearrange("o (k2 ws) l -> k2 o ws l", k2=2)
                        if do_outdma:
                            if old_out and not ring2:
                                eng = nc.sync
                            else:
                                eng = nc.sync if (wc * 2 + m) % 2 == 0 else nc.scalar
                            if stage6 or xt or t0early:
                                for h in range(2):
                                    eng.dma_start(
                                        out=dstf[:, :, 16 * h:16 * h + 16, :],
                                        in_=stages[h][:].rearrange(
                                            "p (ws l) -> p ws l", ws=16))
                            else:
                                eng.dma_start(out=dstf, in_=stage[:].rearrange(
                                    "p (ws l) -> p ws l", ws=32))
    nc.compile()
    return nc


BEST_VARIANT = "full_oldhop0_oldout"


def kernel(**inputs):
    from concourse.bass_utils import run_bass_kernel_spmd

    nc = _build(BEST_VARIANT)
    x = np.ascontiguousarray(np.asarray(inputs["x"], dtype=np.float32))
    shared = {
        "nodevec1": np.ascontiguousarray(np.asarray(inputs["nodevec1"], np.float32)),
        "nodevec2": np.ascontiguousarray(np.asarray(inputs["nodevec2"], np.float32)),
        "w_trans": np.ascontiguousarray(np.asarray(inputs["w_trans"], np.float32)),
        "b_trans": np.ascontiguousarray(np.asarray(inputs["b_trans"], np.float32)),
        "mlp_w": np.ascontiguousarray(np.asarray(inputs["mlp_w"], np.float32)),
        "mlp_b": np.ascontiguousarray(np.asarray(inputs["mlp_b"], np.float32)),
    }
    in_maps = [dict(shared, x=x[c * BL:(c + 1) * BL]) for c in range(NCORES)]
    res = run_bass_kernel_spmd(nc, in_maps, core_ids=list(range(NCORES)))
    return np.concatenate([res.results[i]["out"] for i in range(NCORES)], axis=0)


if __name__ == "__main__":
    rng = np.random.RandomState(0)
    ins = {
        "x": rng.randn(B, C, V, L).astype(np.float32),
        "nodevec1": rng.randn(V, E_DIM).astype(np.float32),
        "nodevec2": rng.randn(E_DIM, V).astype(np.float32),
        "w_trans": (rng.randn(KERNEL_SIZE - 1, E_DIM, E_DIM) * 0.1).astype(np.float32),
        "b_trans": np.zeros((KERNEL_SIZE - 1, E_DIM), np.float32),
        "mlp_w": (rng.randn(C_OUT, 4 * C) / np.sqrt(4 * C)).astype(np.float32),
        "mlp_b": np.zeros((C_OUT,), np.float32),
    }
    out = kernel(**ins)
    print("out", out.shape, out.dtype, float(np.abs(out).max()))

